# revision 26
# baseline (speedup 1.0000x reference)
"""Causal self-attention (single head) on 8 Trainium2 NeuronCores.

Sharding: 8 cores = 4 batches x 2 query-tile parity sets. Core c handles
batch (c % 4). Cores 0-3 take query tiles t in {15,13,...,1} (128 rows
each), cores 4-7 take t in {14,12,...,0}. Attention iteration i=0..7 uses
a fixed causal extent E(i) = 16-2i k-tiles, so a single SPMD program
serves all cores; even-parity cores waste one fully-masked k-tile per
iteration.

Host passes x.T (plus the core's own query columns pre-gathered) and W.T
per core so the device never transposes inputs; operands are fp16 with
f32 PSUM accumulation. Softmax skips max-subtraction (scores/32 stay in a
safe exp range) and gets row sums free via the activation accum_out. All
operands stay SBUF-resident.

Schedule: the G = A x^T critical DMA (at + xk) is split across both DMA
queues in big deadline-ordered chunks; everything else queues behind.
Attention runs largest-extent first through a 3-stage software pipeline
(U(i) -> scores(i+1) -> uT/Z(i)) with PE transposes issued two tiles
ahead of their consumers, so softmax and copy latencies hide behind PE
work and the kernel tail ends on the smallest row. Output is stored as
fp16 (halves store DMA); the host gather upcasts to f32.
"""

import sys

for _p in ("/opt/trn_rl_repo", "/root/.axon_site/_ro/trn_rl_repo"):
    if _p not in sys.path:
        sys.path.append(_p)

import numpy as np

import concourse.bass as bass  # noqa: F401
import concourse.mybir as mybir
import concourse.tile as tile
from concourse import bacc
from concourse.bass_utils import run_bass_kernel_spmd

F32 = mybir.dt.float32
F16 = mybir.dt.float16

BATCH, SEQ, D, P = 4, 2048, 1024, 1024
N_CORES = 8
QT = 128          # query tile rows
KTL = 128         # key tile
NBLK = 512        # matmul moving free dim
ND = D // 128     # 8 d-tiles
NP = P // 128     # 8 p-tiles
NKT = SEQ // KTL  # 16 k-tiles
NQT = 8           # q-tiles per core
SCALE = 1.0 / float(np.sqrt(P))
NEG = -1e9


def _extent(i):
    return 16 - 2 * i


def _chunks(width):
    out = []
    w = width
    while w >= NBLK:
        out.append(NBLK)
        w -= NBLK
    if w:
        assert w == 256, w
        out.append(256)
    return out


def build_program():
    nc = bacc.Bacc("TRN2", target_bir_lowering=False)

    xT = nc.dram_tensor("xT", [D, SEQ], F16, kind="ExternalInput")
    xn = nc.dram_tensor("xn", [SEQ, D], F16, kind="ExternalInput")
    xq_in = nc.dram_tensor("xqcols", [D, NQT * QT], F16, kind="ExternalInput")
    AT = nc.dram_tensor("AT", [D, D], F16, kind="ExternalInput")
    WvT = nc.dram_tensor("WvT", [D, P], F16, kind="ExternalInput")
    mask = nc.dram_tensor("mask", [QT, 256], F32, kind="ExternalInput")
    ident_in = nc.dram_tensor("ident", [128, 128], F16, kind="ExternalInput")
    out = nc.dram_tensor("out", [NQT * QT, P], F16, kind="ExternalOutput")

    # [128, dt, cols] views (partition dim first); full-row reads keep the
    # DMA's contiguous runs at row length (2-4KB), not a sliced 1KB.
    xT_r = xT.rearrange("(dt dp) s -> dp dt s", dp=128)
    xn_r = xn.rearrange("(kt kp) d -> kp kt d", kp=128)
    xq_r = xq_in.rearrange("(dt dp) q -> dp dt q", dp=128)
    at_r = AT.rearrange("(dt dp) d -> dp dt d", dp=128)
    wv_r = WvT.rearrange("(dt dp) p -> dp dt p", dp=128)

    with tile.TileContext(nc) as tc:
        with (
            tc.tile_pool(name="resident", bufs=1) as resident,
            tc.tile_pool(name="wrow", bufs=3) as wrow,
            tc.tile_pool(name="small", bufs=6) as small,
            tc.tile_pool(name="outp", bufs=2) as outp,
            tc.tile_pool(name="p0psum", bufs=2, space="PSUM") as p0psum,
            tc.tile_pool(name="zpsum", bufs=4, space="PSUM") as zpsum,
            tc.tile_pool(name="upsum", bufs=2, space="PSUM") as upsum,
        ):
            kt_sb = resident.tile([128, NP, SEQ], F16)    # G = A x^T [d, k]
            xn_all = resident.tile([128, NKT, D], F16)    # x natural [k, d]
            xq_all = resident.tile([128, ND, NQT * QT], F16)  # x.T q-cols
            xk_all = resident.tile([128, ND, SEQ], F16)   # x.T resident
            at_sb = resident.tile([128, ND, D], F16)      # A^T = Wk^T Wq
            wv_sb = resident.tile([128, ND, P], F16)
            mask_sb = resident.tile([QT, 256], F32)
            ident = resident.tile([128, 128], F16)
            cbias = resident.tile([QT, 1], F32)
            nc.vector.memset(cbias, -4.0)

            # startup loads. The PE queue executes G first, and G's kb-th
            # block needs at (2MB) + xk kb chunk (1MB). DMA issue costs
            # ~0.6us engine time each with ~4 in flight per queue, so use
            # FEW BIG transfers: both queues carry the G-critical path
            # (sync d0-3, scalar d4-7) in deadline order; xq/mask/xn/wv
            # are needed only when attention starts (~G end) and queue
            # strictly behind.
            nc.sync.dma_start(out=at_sb[:, 0:2, :], in_=at_r[:, 0:2, :])
            nc.scalar.dma_start(out=at_sb[:, 4:6, :], in_=at_r[:, 4:6, :])
            nc.sync.dma_start(out=at_sb[:, 2:4, :], in_=at_r[:, 2:4, :])
            nc.scalar.dma_start(out=at_sb[:, 6:ND, :], in_=at_r[:, 6:ND, :])
            for kb in range(SEQ // NBLK):
                s = slice(kb * NBLK, (kb + 1) * NBLK)
                nc.sync.dma_start(out=xk_all[:, 0:4, s], in_=xT_r[:, 0:4, s])
                nc.scalar.dma_start(
                    out=xk_all[:, 4:ND, s], in_=xT_r[:, 4:ND, s])
            nc.scalar.dma_start(out=mask_sb, in_=mask[:, :])
            nc.scalar.dma_start(out=ident, in_=ident_in[:, :])
            nc.scalar.dma_start(out=xq_all, in_=xq_r)
            nc.sync.dma_start(
                out=xn_all[:, 0:NKT // 2, :], in_=xn_r[:, 0:NKT // 2, :])
            nc.sync.dma_start(
                out=xn_all[:, NKT // 2:NKT, :], in_=xn_r[:, NKT // 2:NKT, :])
            nc.scalar.dma_start(out=wv_sb, in_=wv_r)

            # --- G = A x^T and V production ---
            for kb in range(SEQ // NBLK):
                for pt in range(NP):
                    ps = p0psum.tile([128, NBLK], F32, tag="p0")
                    for d in range(ND):
                        nc.tensor.matmul(
                            ps,
                            at_sb[:, d, pt * 128:(pt + 1) * 128],
                            xk_all[:, d, kb * NBLK:(kb + 1) * NBLK],
                            start=(d == 0),
                            stop=(d == ND - 1),
                        )
                    nc.scalar.copy(kt_sb[:, pt, kb * NBLK:(kb + 1) * NBLK], ps)

            # --- attention, largest extent first, software-pipelined:
            # scores(i+1) is issued on the PE queue before U/Z(i), so the
            # softmax (vector mask + scalar exp) of each row hides behind
            # PE work instead of bubbling, and the kernel tail ends on the
            # smallest row (ext=2). ---
            def emit_scores(i):
                ext = _extent(i)
                width = ext * KTL
                chunks = _chunks(width)
                s_ps = []
                off = 0
                for cw in chunks:
                    ps_full = p0psum.tile([QT, NBLK], F32, tag="p0")
                    ps = ps_full[:, :cw]
                    for pt in range(NP):
                        nc.tensor.matmul(
                            ps,
                            xq_all[:, pt, i * QT:(i + 1) * QT],
                            kt_sb[:, pt, off:off + cw],
                            start=(pt == 0),
                            stop=(pt == NP - 1),
                        )
                    s_ps.append((ps, off, cw))
                    off += cw

                # additive causal mask on the last 256 columns of the row
                last_ps, _, last_w = s_ps[-1]
                nc.vector.tensor_add(
                    last_ps[:, last_w - 256:last_w],
                    last_ps[:, last_w - 256:last_w],
                    mask_sb,
                )

                # exp((s + m) * scale) -> fp16 weights row; row sums free
                w_sb = wrow.tile([QT, width], F16, tag="w")
                lparts = small.tile([QT, len(chunks)], F32, tag="lp")
                for ci, (ps, off_c, cw) in enumerate(s_ps):
                    nc.scalar.activation(
                        w_sb[:, off_c:off_c + cw],
                        ps,
                        mybir.ActivationFunctionType.Exp,
                        scale=SCALE,
                        bias=cbias,
                        accum_out=lparts[:, ci:ci + 1],
                    )

                lsum = small.tile([QT, 1], F32, tag="ls")
                nc.vector.reduce_sum(lsum, lparts, axis=mybir.AxisListType.X)
                rl = small.tile([QT, 1], F32, tag="rl")
                nc.vector.reciprocal(rl, lsum)
                return w_sb, rl

            def emit_u(i, w_sb):
                ext = _extent(i)
                # W^T via the DMA transpose XBAR (SBUF->SBUF, off the PE),
                # issued two k-tiles ahead on alternating HWDGE queues.
                # U^T[d, q] is then computed directly with xn as the
                # stationary operand — no PE transposes anywhere. The 8
                # d-tile accumulators pack into two PSUM banks; start=True
                # fires only on each bank's first write (PSUM zeroing is
                # 2KB-region granular), the other slices accumulate onto
                # the zeroed bank.
                up0 = upsum.tile([128, NBLK], F32, tag="up")
                up1 = upsum.tile([128, NBLK], F32, tag="up")
                wTs = {}

                def wtp_one(kt):
                    wT = small.tile([128, 128], F16, tag="wT")
                    q = nc.sync if kt % 2 == 0 else nc.scalar
                    q.dma_start(
                        out=wT, in_=w_sb[:, kt * 128:(kt + 1) * 128],
                        transpose=True)
                    wTs[kt] = wT

                wtp_one(0)
                if ext > 1:
                    wtp_one(1)
                for kt in range(ext):
                    if kt + 2 < ext:
                        wtp_one(kt + 2)
                    for dt in range(ND):
                        ps = up0 if dt < 4 else up1
                        c = (dt % 4) * 128
                        nc.tensor.matmul(
                            ps[:, c:c + 128],
                            xn_all[:, kt, dt * 128:(dt + 1) * 128],
                            wTs[kt],
                            start=(kt == 0 and dt % 4 == 0),
                            stop=(kt == ext - 1),
                            skip_group_check=True,
                        )
                uT_sb = small.tile([128, ND * 128], F16, tag="uT")
                nc.scalar.copy(uT_sb[:, 0:NBLK], up0)
                nc.vector.tensor_copy(uT_sb[:, NBLK:ND * 128], up1)
                return uT_sb

            def emit_z(i, uT_sb, rl):
                z0 = zpsum.tile([QT, NBLK], F32, tag="z")
                z1 = zpsum.tile([QT, NBLK], F32, tag="z")
                for dt in range(ND):
                    uT = uT_sb[:, dt * 128:(dt + 1) * 128]
                    nc.tensor.matmul(
                        z0, uT, wv_sb[:, dt, 0:NBLK],
                        start=(dt == 0), stop=(dt == ND - 1),
                    )
                    nc.tensor.matmul(
                        z1, uT, wv_sb[:, dt, NBLK:P],
                        start=(dt == 0), stop=(dt == ND - 1),
                    )

                o_sb = outp.tile([QT, P], F16, tag="o")
                nc.vector.tensor_scalar_mul(o_sb[:, 0:NBLK], z0, rl)
                nc.sync.dma_start(
                    out=out[i * QT:(i + 1) * QT, 0:NBLK], in_=o_sb[:, 0:NBLK])
                nc.vector.tensor_scalar_mul(o_sb[:, NBLK:P], z1, rl)
                nc.scalar.dma_start(
                    out=out[i * QT:(i + 1) * QT, NBLK:P], in_=o_sb[:, NBLK:P])

            # 3-stage pipeline: U(i) then scores(i+1) then uT/Z(i), so both
            # the exp latency of row i+1 and the u-copy latency of row i
            # hide behind PE work. The last two rows' scores (ext 4 and 2)
            # are too short to hide their own softmax, so they are issued
            # two ahead, during row 5's U/Z.
            sc = {}
            sc[0] = emit_scores(0)
            for idx in range(NQT):
                u_sb = emit_u(idx, sc[idx][0])
                if idx + 1 <= 5:
                    sc[idx + 1] = emit_scores(idx + 1)
                elif idx + 1 == 6:
                    sc[6] = emit_scores(6)
                    sc[7] = emit_scores(7)
                emit_z(idx, u_sb, sc[idx][1])

    nc.compile()
    return nc


def _tiles_for_core(c):
    """Global 128-row query-tile indices, in program order i=0..7."""
    return [(15 - 2 * i) if c < 4 else (14 - 2 * i) for i in range(NQT)]


def _host_prep(inputs, Wq, Wk, Wv):
    x = np.asarray(inputs, dtype=np.float32)
    Wqf = np.asarray(Wq, dtype=np.float32)
    Wkf = np.asarray(Wk, dtype=np.float32)
    # scores = x (Wq^T Wk) x^T; device stationary wants the transpose
    ATm = np.ascontiguousarray((Wkf.T @ Wqf).astype(np.float16))
    WvT = np.ascontiguousarray(
        np.asarray(Wv, dtype=np.float32).T.astype(np.float16))

    qi = np.arange(QT)[:, None]
    ki = np.arange(128)[None, :]
    tri = np.where(qi >= ki, 0.0, NEG).astype(np.float32)
    mask_hi = np.concatenate([np.zeros((QT, 128), np.float32), tri], axis=1)
    mask_lo = np.concatenate(
        [tri, np.full((QT, 128), NEG, np.float32)], axis=1)

    in_maps = []
    xT_cache = {}
    for c in range(N_CORES):
        b = c % 4
        if b not in xT_cache:
            xT_cache[b] = np.ascontiguousarray(x[b].T.astype(np.float16))
        xTb = xT_cache[b]
        cols = np.concatenate(
            [xTb[:, t * QT:(t + 1) * QT] for t in _tiles_for_core(c)], axis=1)
        in_maps.append({
            "xT": xTb,
            "xn": np.ascontiguousarray(x[b].astype(np.float16)),
            "xqcols": np.ascontiguousarray(cols),
            "AT": ATm,
            "WvT": WvT,
            "mask": mask_hi if c < 4 else mask_lo,
            "ident": np.eye(128, dtype=np.float16),
        })
    return in_maps


def _host_gather(results):
    Z = np.empty((BATCH, SEQ, P), dtype=np.float32)
    for c in range(N_CORES):
        b = c % 4
        o = results[c]["out"]
        for i, t in enumerate(_tiles_for_core(c)):
            Z[b, t * QT:(t + 1) * QT, :] = o[i * QT:(i + 1) * QT, :]
    return Z


_NC_CACHE = None


def kernel(inputs, Wq, Wk, Wv):
    global _NC_CACHE
    if _NC_CACHE is None:
        _NC_CACHE = build_program()
    in_maps = _host_prep(inputs, Wq, Wk, Wv)
    # The first execution after a fresh compile occasionally hits a
    # transient NRT_EXEC_UNIT_UNRECOVERABLE; a retry reliably succeeds.
    last_err = None
    Z = None
    for _ in range(3):
        try:
            res = run_bass_kernel_spmd(
                _NC_CACHE, in_maps, list(range(N_CORES)))
            Z = _host_gather(res.results)
            if np.isfinite(Z).all():
                return Z
        except Exception as e:  # noqa: BLE001
            last_err = e
    if Z is not None:
        return Z
    raise last_err



# revision 28
# speedup vs baseline: 1.1959x; 1.1959x over previous
"""Causal self-attention (single head) on 8 Trainium2 NeuronCores.

Sharding: 8 cores = 4 batches x 2 query-tile parity sets. Core c handles
batch (c % 4). Cores 0-3 take query tiles t in {15,13,...,1} (128 rows
each), cores 4-7 take t in {14,12,...,0}. Attention iteration i=0..7 uses
a fixed causal extent E(i) = 16-2i k-tiles, so a single SPMD program
serves all cores; even-parity cores waste one fully-masked k-tile per
iteration.

Host passes x.T (plus the core's own query columns pre-gathered) and W.T
per core so the device never transposes inputs; operands are fp16 with
f32 PSUM accumulation. Softmax skips max-subtraction (scores/32 stay in a
safe exp range) and gets row sums free via the activation accum_out. All
operands stay SBUF-resident.

Schedule: the G = A x^T critical DMA (at + xk) is split across both DMA
queues in big deadline-ordered chunks; everything else queues behind.
Attention runs largest-extent first through a 3-stage software pipeline
(U(i) -> scores(i+1) -> uT/Z(i)) with PE transposes issued two tiles
ahead of their consumers, so softmax and copy latencies hide behind PE
work and the kernel tail ends on the smallest row. Output is stored as
fp16 (halves store DMA); the host gather upcasts to f32.
"""

import sys

for _p in ("/opt/trn_rl_repo", "/root/.axon_site/_ro/trn_rl_repo"):
    if _p not in sys.path:
        sys.path.append(_p)

import numpy as np

import concourse.bass as bass  # noqa: F401
import concourse.mybir as mybir
import concourse.tile as tile
from concourse import bacc
from concourse.bass_utils import run_bass_kernel_spmd

F32 = mybir.dt.float32
F16 = mybir.dt.float16

BATCH, SEQ, D, P = 4, 2048, 1024, 1024
N_CORES = 8
QT = 128          # query tile rows
KTL = 128         # key tile
NBLK = 512        # matmul moving free dim
ND = D // 128     # 8 d-tiles
NP = P // 128     # 8 p-tiles
NKT = SEQ // KTL  # 16 k-tiles
NQT = 8           # q-tiles per core
SCALE = 1.0 / float(np.sqrt(P))
NEG = -1e9


def _extent(i):
    return 16 - 2 * i


def _chunks(width):
    out = []
    w = width
    while w >= NBLK:
        out.append(NBLK)
        w -= NBLK
    if w:
        assert w == 256, w
        out.append(256)
    return out


def build_program():
    nc = bacc.Bacc("TRN2", target_bir_lowering=False)

    xT = nc.dram_tensor("xT", [D, SEQ], F16, kind="ExternalInput")
    xn = nc.dram_tensor("xn", [SEQ, D], F16, kind="ExternalInput")
    xq_in = nc.dram_tensor("xqcols", [D, NQT * QT], F16, kind="ExternalInput")
    AT = nc.dram_tensor("AT", [D, D], F16, kind="ExternalInput")
    WvT = nc.dram_tensor("WvT", [D, P], F16, kind="ExternalInput")
    mask = nc.dram_tensor("mask", [QT, 256], F32, kind="ExternalInput")
    ident_in = nc.dram_tensor("ident", [128, 128], F16, kind="ExternalInput")
    out = nc.dram_tensor("out", [NQT * QT, P], F16, kind="ExternalOutput")

    # [128, dt, cols] views (partition dim first); full-row reads keep the
    # DMA's contiguous runs at row length (2-4KB), not a sliced 1KB.
    xT_r = xT.rearrange("(dt dp) s -> dp dt s", dp=128)
    xn_r = xn.rearrange("(kt kp) d -> kp kt d", kp=128)
    xq_r = xq_in.rearrange("(dt dp) q -> dp dt q", dp=128)
    at_r = AT.rearrange("(dt dp) d -> dp dt d", dp=128)
    wv_r = WvT.rearrange("(dt dp) p -> dp dt p", dp=128)

    with tile.TileContext(nc) as tc:
        with (
            tc.tile_pool(name="resident", bufs=1) as resident,
            tc.tile_pool(name="wrow", bufs=3) as wrow,
            tc.tile_pool(name="small", bufs=6) as small,
            tc.tile_pool(name="outp", bufs=2) as outp,
            tc.tile_pool(name="p0psum", bufs=2, space="PSUM") as p0psum,
            tc.tile_pool(name="zpsum", bufs=4, space="PSUM") as zpsum,
            tc.tile_pool(name="tpsum", bufs=2, space="PSUM") as tpsum,
        ):
            kt_sb = resident.tile([128, NP, SEQ], F16)    # G = A x^T [d, k]
            xn_all = resident.tile([128, NKT, D], F16)    # x natural [k, d]
            xq_all = resident.tile([128, ND, NQT * QT], F16)  # x.T q-cols
            xk_all = resident.tile([128, ND, SEQ], F16)   # x.T resident
            at_sb = resident.tile([128, ND, D], F16)      # A^T = Wk^T Wq
            wv_sb = resident.tile([128, ND, P], F16)
            mask_sb = resident.tile([QT, 256], F32)
            ident = resident.tile([128, 128], F16)
            cbias = resident.tile([QT, 1], F32)
            nc.vector.memset(cbias, -4.0)

            # startup loads. The PE queue executes G first, and G's kb-th
            # block needs at (2MB) + xk kb chunk (1MB). DMA issue costs
            # ~0.6us engine time each with ~4 in flight per queue, so use
            # FEW BIG transfers: both queues carry the G-critical path
            # (sync d0-3, scalar d4-7) in deadline order; xq/mask/xn/wv
            # are needed only when attention starts (~G end) and queue
            # strictly behind.
            nc.sync.dma_start(out=at_sb[:, 0:2, :], in_=at_r[:, 0:2, :])
            nc.scalar.dma_start(out=at_sb[:, 4:6, :], in_=at_r[:, 4:6, :])
            nc.sync.dma_start(out=at_sb[:, 2:4, :], in_=at_r[:, 2:4, :])
            nc.scalar.dma_start(out=at_sb[:, 6:ND, :], in_=at_r[:, 6:ND, :])
            for kb in range(SEQ // NBLK):
                s = slice(kb * NBLK, (kb + 1) * NBLK)
                nc.sync.dma_start(out=xk_all[:, 0:4, s], in_=xT_r[:, 0:4, s])
                nc.scalar.dma_start(
                    out=xk_all[:, 4:ND, s], in_=xT_r[:, 4:ND, s])
            nc.scalar.dma_start(out=mask_sb, in_=mask[:, :])
            nc.scalar.dma_start(out=ident, in_=ident_in[:, :])
            nc.scalar.dma_start(out=xq_all, in_=xq_r)
            nc.sync.dma_start(
                out=xn_all[:, 0:NKT // 2, :], in_=xn_r[:, 0:NKT // 2, :])
            nc.sync.dma_start(
                out=xn_all[:, NKT // 2:NKT, :], in_=xn_r[:, NKT // 2:NKT, :])
            nc.scalar.dma_start(out=wv_sb, in_=wv_r)

            # --- G = A x^T and V production ---
            for kb in range(SEQ // NBLK):
                for pt in range(NP):
                    ps = p0psum.tile([128, NBLK], F32, tag="p0")
                    for d in range(ND):
                        nc.tensor.matmul(
                            ps,
                            at_sb[:, d, pt * 128:(pt + 1) * 128],
                            xk_all[:, d, kb * NBLK:(kb + 1) * NBLK],
                            start=(d == 0),
                            stop=(d == ND - 1),
                        )
                    nc.scalar.copy(kt_sb[:, pt, kb * NBLK:(kb + 1) * NBLK], ps)

            # --- attention, largest extent first, software-pipelined:
            # scores(i+1) is issued on the PE queue before U/Z(i), so the
            # softmax (vector mask + scalar exp) of each row hides behind
            # PE work instead of bubbling, and the kernel tail ends on the
            # smallest row (ext=2). ---
            def emit_scores(i):
                ext = _extent(i)
                width = ext * KTL
                chunks = _chunks(width)
                s_ps = []
                off = 0
                for cw in chunks:
                    ps_full = p0psum.tile([QT, NBLK], F32, tag="p0")
                    ps = ps_full[:, :cw]
                    for pt in range(NP):
                        nc.tensor.matmul(
                            ps,
                            xq_all[:, pt, i * QT:(i + 1) * QT],
                            kt_sb[:, pt, off:off + cw],
                            start=(pt == 0),
                            stop=(pt == NP - 1),
                        )
                    s_ps.append((ps, off, cw))
                    off += cw

                # additive causal mask on the last 256 columns of the row
                last_ps, _, last_w = s_ps[-1]
                nc.vector.tensor_add(
                    last_ps[:, last_w - 256:last_w],
                    last_ps[:, last_w - 256:last_w],
                    mask_sb,
                )

                # exp((s + m) * scale) -> fp16 weights row; row sums free
                w_sb = wrow.tile([QT, width], F16, tag="w")
                lparts = small.tile([QT, len(chunks)], F32, tag="lp")
                for ci, (ps, off_c, cw) in enumerate(s_ps):
                    nc.scalar.activation(
                        w_sb[:, off_c:off_c + cw],
                        ps,
                        mybir.ActivationFunctionType.Exp,
                        scale=SCALE,
                        bias=cbias,
                        accum_out=lparts[:, ci:ci + 1],
                    )

                lsum = small.tile([QT, 1], F32, tag="ls")
                nc.vector.reduce_sum(lsum, lparts, axis=mybir.AxisListType.X)
                rl = small.tile([QT, 1], F32, tag="rl")
                nc.vector.reciprocal(rl, lsum)
                return w_sb, rl

            def emit_u(i, w_sb):
                ext = _extent(i)
                # U = W x  (transpose each weight block on PE, two k-tiles
                # ahead of the consuming matmuls)
                u0 = zpsum.tile([QT, NBLK], F32, tag="z")
                u1 = zpsum.tile([QT, NBLK], F32, tag="z")
                wTs = {}

                def wtp_one(kt):
                    tp = tpsum.tile([128, 128], F16, tag="tp")
                    nc.tensor.transpose(
                        tp, w_sb[:, kt * 128:(kt + 1) * 128], ident)
                    wT = small.tile([128, 128], F16, tag="wT")
                    nc.vector.tensor_copy(wT, tp)
                    wTs[kt] = wT

                wtp_one(0)
                if ext > 1:
                    wtp_one(1)
                for kt in range(ext):
                    if kt + 2 < ext:
                        wtp_one(kt + 2)
                    nc.tensor.matmul(
                        u0, wTs[kt], xn_all[:, kt, 0:NBLK],
                        start=(kt == 0), stop=(kt == ext - 1),
                    )
                    nc.tensor.matmul(
                        u1, wTs[kt], xn_all[:, kt, NBLK:D],
                        start=(kt == 0), stop=(kt == ext - 1),
                    )
                u_sb = wrow.tile([QT, D], F16, tag="u")
                nc.scalar.copy(u_sb[:, 0:NBLK], u0)
                nc.vector.tensor_copy(u_sb[:, NBLK:D], u1)
                return u_sb

            def emit_z(i, u_sb, rl):
                # Z = U Wv^T  (U transposed per d-tile on PE, two tiles
                # ahead of the Z accumulation)
                uT_sb = small.tile([128, ND, 128], F16, tag="uT")

                def tp_one(dt):
                    tpu = tpsum.tile([128, 128], F16, tag="tp")
                    nc.tensor.transpose(
                        tpu, u_sb[:, dt * 128:(dt + 1) * 128], ident)
                    nc.vector.tensor_copy(uT_sb[:, dt, :], tpu)

                tp_one(0)
                tp_one(1)
                z0 = zpsum.tile([QT, NBLK], F32, tag="z")
                z1 = zpsum.tile([QT, NBLK], F32, tag="z")
                for dt in range(ND):
                    if dt + 2 < ND:
                        tp_one(dt + 2)
                    nc.tensor.matmul(
                        z0, uT_sb[:, dt, :], wv_sb[:, dt, 0:NBLK],
                        start=(dt == 0), stop=(dt == ND - 1),
                    )
                    nc.tensor.matmul(
                        z1, uT_sb[:, dt, :], wv_sb[:, dt, NBLK:P],
                        start=(dt == 0), stop=(dt == ND - 1),
                    )

                o_sb = outp.tile([QT, P], F16, tag="o")
                nc.vector.tensor_scalar_mul(o_sb[:, 0:NBLK], z0, rl)
                nc.sync.dma_start(
                    out=out[i * QT:(i + 1) * QT, 0:NBLK], in_=o_sb[:, 0:NBLK])
                nc.vector.tensor_scalar_mul(o_sb[:, NBLK:P], z1, rl)
                nc.scalar.dma_start(
                    out=out[i * QT:(i + 1) * QT, NBLK:P], in_=o_sb[:, NBLK:P])

            # 3-stage pipeline: U(i) then scores(i+1) then uT/Z(i), so both
            # the exp latency of row i+1 and the u-copy latency of row i
            # hide behind PE work. The last two rows' scores (ext 4 and 2)
            # are too short to hide their own softmax, so they are issued
            # two ahead, during row 5's U/Z.
            sc = {}
            sc[0] = emit_scores(0)
            for idx in range(NQT):
                u_sb = emit_u(idx, sc[idx][0])
                if idx + 1 <= 5:
                    sc[idx + 1] = emit_scores(idx + 1)
                elif idx + 1 == 6:
                    sc[6] = emit_scores(6)
                    sc[7] = emit_scores(7)
                emit_z(idx, u_sb, sc[idx][1])

    nc.compile()
    return nc


def _tiles_for_core(c):
    """Global 128-row query-tile indices, in program order i=0..7."""
    return [(15 - 2 * i) if c < 4 else (14 - 2 * i) for i in range(NQT)]


def _host_prep(inputs, Wq, Wk, Wv):
    x = np.asarray(inputs, dtype=np.float32)
    Wqf = np.asarray(Wq, dtype=np.float32)
    Wkf = np.asarray(Wk, dtype=np.float32)
    # scores = x (Wq^T Wk) x^T; device stationary wants the transpose
    ATm = np.ascontiguousarray((Wkf.T @ Wqf).astype(np.float16))
    WvT = np.ascontiguousarray(
        np.asarray(Wv, dtype=np.float32).T.astype(np.float16))

    qi = np.arange(QT)[:, None]
    ki = np.arange(128)[None, :]
    tri = np.where(qi >= ki, 0.0, NEG).astype(np.float32)
    mask_hi = np.concatenate([np.zeros((QT, 128), np.float32), tri], axis=1)
    mask_lo = np.concatenate(
        [tri, np.full((QT, 128), NEG, np.float32)], axis=1)

    in_maps = []
    xT_cache = {}
    for c in range(N_CORES):
        b = c % 4
        if b not in xT_cache:
            xT_cache[b] = np.ascontiguousarray(x[b].T.astype(np.float16))
        xTb = xT_cache[b]
        cols = np.concatenate(
            [xTb[:, t * QT:(t + 1) * QT] for t in _tiles_for_core(c)], axis=1)
        in_maps.append({
            "xT": xTb,
            "xn": np.ascontiguousarray(x[b].astype(np.float16)),
            "xqcols": np.ascontiguousarray(cols),
            "AT": ATm,
            "WvT": WvT,
            "mask": mask_hi if c < 4 else mask_lo,
            "ident": np.eye(128, dtype=np.float16),
        })
    return in_maps


def _host_gather(results):
    Z = np.empty((BATCH, SEQ, P), dtype=np.float32)
    for c in range(N_CORES):
        b = c % 4
        o = results[c]["out"]
        for i, t in enumerate(_tiles_for_core(c)):
            Z[b, t * QT:(t + 1) * QT, :] = o[i * QT:(i + 1) * QT, :]
    return Z


_NC_CACHE = None


def kernel(inputs, Wq, Wk, Wv):
    global _NC_CACHE
    if _NC_CACHE is None:
        _NC_CACHE = build_program()
    in_maps = _host_prep(inputs, Wq, Wk, Wv)
    # The first execution after a fresh compile occasionally hits a
    # transient NRT_EXEC_UNIT_UNRECOVERABLE; a retry reliably succeeds.
    last_err = None
    Z = None
    for _ in range(3):
        try:
            res = run_bass_kernel_spmd(
                _NC_CACHE, in_maps, list(range(N_CORES)))
            Z = _host_gather(res.results)
            if np.isfinite(Z).all():
                return Z
        except Exception as e:  # noqa: BLE001
            last_err = e
    if Z is not None:
        return Z
    raise last_err



# revision 29
# speedup vs baseline: 1.2079x; 1.0101x over previous
"""Causal self-attention (single head) on 8 Trainium2 NeuronCores.

Sharding: 8 cores = 4 batches x 2 query-tile parity sets. Core c handles
batch (c % 4). Cores 0-3 take query tiles t in {15,13,...,1} (128 rows
each), cores 4-7 take t in {14,12,...,0}. Attention iteration i=0..7 uses
a fixed causal extent E(i) = 16-2i k-tiles, so a single SPMD program
serves all cores; even-parity cores waste one fully-masked k-tile per
iteration.

Host passes x.T (plus the core's own query columns pre-gathered) and W.T
per core so the device never transposes inputs; operands are fp16 with
f32 PSUM accumulation. Softmax skips max-subtraction (scores/32 stay in a
safe exp range) and gets row sums free via the activation accum_out. All
operands stay SBUF-resident.

Schedule: the G = A x^T critical DMA (at + xk) is split across both DMA
queues in big deadline-ordered chunks; everything else queues behind.
Attention runs largest-extent first through a 3-stage software pipeline
(U(i) -> scores(i+1) -> uT/Z(i)) with PE transposes issued two tiles
ahead of their consumers, so softmax and copy latencies hide behind PE
work and the kernel tail ends on the smallest row. Output is stored as
fp16 (halves store DMA); the host gather upcasts to f32.
"""

import sys

for _p in ("/opt/trn_rl_repo", "/root/.axon_site/_ro/trn_rl_repo"):
    if _p not in sys.path:
        sys.path.append(_p)

import numpy as np

import concourse.bass as bass  # noqa: F401
import concourse.mybir as mybir
import concourse.tile as tile
from concourse import bacc
from concourse.bass_utils import run_bass_kernel_spmd

F32 = mybir.dt.float32
F16 = mybir.dt.float16

BATCH, SEQ, D, P = 4, 2048, 1024, 1024
N_CORES = 8
QT = 128          # query tile rows
KTL = 128         # key tile
NBLK = 512        # matmul moving free dim
ND = D // 128     # 8 d-tiles
NP = P // 128     # 8 p-tiles
NKT = SEQ // KTL  # 16 k-tiles
NQT = 8           # q-tiles per core
SCALE = 1.0 / float(np.sqrt(P))
NEG = -1e9


def _extent(i):
    return 16 - 2 * i


def _chunks(width):
    out = []
    w = width
    while w >= NBLK:
        out.append(NBLK)
        w -= NBLK
    if w:
        assert w == 256, w
        out.append(256)
    return out


def build_program():
    nc = bacc.Bacc("TRN2", target_bir_lowering=False)

    xT = nc.dram_tensor("xT", [D, SEQ], F16, kind="ExternalInput")
    xn = nc.dram_tensor("xn", [SEQ, D], F16, kind="ExternalInput")
    xq_in = nc.dram_tensor("xqcols", [D, NQT * QT], F16, kind="ExternalInput")
    AT = nc.dram_tensor("AT", [D, D], F16, kind="ExternalInput")
    WvT = nc.dram_tensor("WvT", [D, P], F16, kind="ExternalInput")
    mask = nc.dram_tensor("mask", [QT, 256], F32, kind="ExternalInput")
    ident_in = nc.dram_tensor("ident", [128, 128], F16, kind="ExternalInput")
    out = nc.dram_tensor("out", [NQT * QT, P], F16, kind="ExternalOutput")

    # [128, dt, cols] views (partition dim first); full-row reads keep the
    # DMA's contiguous runs at row length (2-4KB), not a sliced 1KB.
    xT_r = xT.rearrange("(dt dp) s -> dp dt s", dp=128)
    xn_r = xn.rearrange("(kt kp) d -> kp kt d", kp=128)
    xq_r = xq_in.rearrange("(dt dp) q -> dp dt q", dp=128)
    at_r = AT.rearrange("(dt dp) d -> dp dt d", dp=128)
    wv_r = WvT.rearrange("(dt dp) p -> dp dt p", dp=128)

    with tile.TileContext(nc) as tc:
        with (
            tc.tile_pool(name="resident", bufs=1) as resident,
            tc.tile_pool(name="wrow", bufs=3) as wrow,
            tc.tile_pool(name="small", bufs=6) as small,
            tc.tile_pool(name="outp", bufs=2) as outp,
            tc.tile_pool(name="p0psum", bufs=2, space="PSUM") as p0psum,
            tc.tile_pool(name="zpsum", bufs=4, space="PSUM") as zpsum,
            tc.tile_pool(name="tpsum", bufs=2, space="PSUM") as tpsum,
        ):
            kt_sb = resident.tile([128, NP, SEQ], F16)    # G = A x^T [d, k]
            xn_all = resident.tile([128, NKT, D], F16)    # x natural [k, d]
            xq_all = resident.tile([128, ND, NQT * QT], F16)  # x.T q-cols
            xk_all = resident.tile([128, ND, SEQ], F16)   # x.T resident
            at_sb = resident.tile([128, ND, D], F16)      # A^T = Wk^T Wq
            wv_sb = resident.tile([128, ND, P], F16)
            mask_sb = resident.tile([QT, 256], F32)
            ident = resident.tile([128, 128], F16)
            cbias = resident.tile([QT, 1], F32)
            nc.vector.memset(cbias, -4.0)

            # startup loads. The PE queue executes G first, and G's kb-th
            # block needs at (2MB) + xk kb chunk (1MB). DMA issue costs
            # ~0.6us engine time each with ~4 in flight per queue, so use
            # FEW BIG transfers: both queues carry the G-critical path
            # (sync d0-3, scalar d4-7) in deadline order; xq/mask/xn/wv
            # are needed only when attention starts (~G end) and queue
            # strictly behind.
            nc.sync.dma_start(out=at_sb[:, 0:2, :], in_=at_r[:, 0:2, :])
            nc.scalar.dma_start(out=at_sb[:, 4:6, :], in_=at_r[:, 4:6, :])
            nc.sync.dma_start(out=at_sb[:, 2:4, :], in_=at_r[:, 2:4, :])
            nc.scalar.dma_start(out=at_sb[:, 6:ND, :], in_=at_r[:, 6:ND, :])
            for kb in range(SEQ // NBLK):
                s = slice(kb * NBLK, (kb + 1) * NBLK)
                nc.sync.dma_start(out=xk_all[:, 0:4, s], in_=xT_r[:, 0:4, s])
                nc.scalar.dma_start(
                    out=xk_all[:, 4:ND, s], in_=xT_r[:, 4:ND, s])
            nc.scalar.dma_start(out=mask_sb, in_=mask[:, :])
            nc.scalar.dma_start(out=ident, in_=ident_in[:, :])
            nc.scalar.dma_start(out=xq_all, in_=xq_r)
            nc.sync.dma_start(
                out=xn_all[:, 0:NKT // 2, :], in_=xn_r[:, 0:NKT // 2, :])
            nc.sync.dma_start(
                out=xn_all[:, NKT // 2:NKT, :], in_=xn_r[:, NKT // 2:NKT, :])
            nc.scalar.dma_start(out=wv_sb, in_=wv_r)

            # --- G = A x^T and V production ---
            for kb in range(SEQ // NBLK):
                for pt in range(NP):
                    ps = p0psum.tile([128, NBLK], F32, tag="p0")
                    for d in range(ND):
                        nc.tensor.matmul(
                            ps,
                            at_sb[:, d, pt * 128:(pt + 1) * 128],
                            xk_all[:, d, kb * NBLK:(kb + 1) * NBLK],
                            start=(d == 0),
                            stop=(d == ND - 1),
                        )
                    dst = kt_sb[:, pt, kb * NBLK:(kb + 1) * NBLK]
                    if pt % 2 == 0:
                        nc.scalar.copy(dst, ps)
                    else:
                        nc.vector.tensor_copy(dst, ps)

            # --- attention, largest extent first, software-pipelined:
            # scores(i+1) is issued on the PE queue before U/Z(i), so the
            # softmax (vector mask + scalar exp) of each row hides behind
            # PE work instead of bubbling, and the kernel tail ends on the
            # smallest row (ext=2). ---
            def emit_scores(i):
                ext = _extent(i)
                width = ext * KTL
                chunks = _chunks(width)
                s_ps = []
                off = 0
                for cw in chunks:
                    ps_full = p0psum.tile([QT, NBLK], F32, tag="p0")
                    ps = ps_full[:, :cw]
                    for pt in range(NP):
                        nc.tensor.matmul(
                            ps,
                            xq_all[:, pt, i * QT:(i + 1) * QT],
                            kt_sb[:, pt, off:off + cw],
                            start=(pt == 0),
                            stop=(pt == NP - 1),
                        )
                    s_ps.append((ps, off, cw))
                    off += cw

                # additive causal mask on the last 256 columns of the row
                last_ps, _, last_w = s_ps[-1]
                nc.vector.tensor_add(
                    last_ps[:, last_w - 256:last_w],
                    last_ps[:, last_w - 256:last_w],
                    mask_sb,
                )

                # exp((s + m) * scale) -> fp16 weights row; row sums free
                w_sb = wrow.tile([QT, width], F16, tag="w")
                lparts = small.tile([QT, len(chunks)], F32, tag="lp")
                for ci, (ps, off_c, cw) in enumerate(s_ps):
                    nc.scalar.activation(
                        w_sb[:, off_c:off_c + cw],
                        ps,
                        mybir.ActivationFunctionType.Exp,
                        scale=SCALE,
                        bias=cbias,
                        accum_out=lparts[:, ci:ci + 1],
                    )

                lsum = small.tile([QT, 1], F32, tag="ls")
                nc.vector.reduce_sum(lsum, lparts, axis=mybir.AxisListType.X)
                rl = small.tile([QT, 1], F32, tag="rl")
                nc.vector.reciprocal(rl, lsum)
                return w_sb, rl

            def emit_u(i, w_sb):
                ext = _extent(i)
                # U = W x  (transpose each weight block on PE, two k-tiles
                # ahead of the consuming matmuls)
                u0 = zpsum.tile([QT, NBLK], F32, tag="z")
                u1 = zpsum.tile([QT, NBLK], F32, tag="z")
                wTs = {}

                def wtp_one(kt):
                    tp = tpsum.tile([128, 128], F16, tag="tp")
                    nc.tensor.transpose(
                        tp, w_sb[:, kt * 128:(kt + 1) * 128], ident)
                    wT = small.tile([128, 128], F16, tag="wT")
                    nc.vector.tensor_copy(wT, tp)
                    wTs[kt] = wT

                wtp_one(0)
                if ext > 1:
                    wtp_one(1)
                for kt in range(ext):
                    if kt + 2 < ext:
                        wtp_one(kt + 2)
                    nc.tensor.matmul(
                        u0, wTs[kt], xn_all[:, kt, 0:NBLK],
                        start=(kt == 0), stop=(kt == ext - 1),
                    )
                    nc.tensor.matmul(
                        u1, wTs[kt], xn_all[:, kt, NBLK:D],
                        start=(kt == 0), stop=(kt == ext - 1),
                    )
                u_sb = wrow.tile([QT, D], F16, tag="u")
                nc.scalar.copy(u_sb[:, 0:NBLK], u0)
                nc.vector.tensor_copy(u_sb[:, NBLK:D], u1)
                return u_sb

            def emit_z(i, u_sb, rl):
                # Z = U Wv^T  (U transposed per d-tile on PE, two tiles
                # ahead of the Z accumulation)
                uT_sb = small.tile([128, ND, 128], F16, tag="uT")

                def tp_one(dt):
                    tpu = tpsum.tile([128, 128], F16, tag="tp")
                    nc.tensor.transpose(
                        tpu, u_sb[:, dt * 128:(dt + 1) * 128], ident)
                    nc.vector.tensor_copy(uT_sb[:, dt, :], tpu)

                tp_one(0)
                tp_one(1)
                z0 = zpsum.tile([QT, NBLK], F32, tag="z")
                z1 = zpsum.tile([QT, NBLK], F32, tag="z")
                for dt in range(ND):
                    if dt + 2 < ND:
                        tp_one(dt + 2)
                    nc.tensor.matmul(
                        z0, uT_sb[:, dt, :], wv_sb[:, dt, 0:NBLK],
                        start=(dt == 0), stop=(dt == ND - 1),
                    )
                    nc.tensor.matmul(
                        z1, uT_sb[:, dt, :], wv_sb[:, dt, NBLK:P],
                        start=(dt == 0), stop=(dt == ND - 1),
                    )

                o_sb = outp.tile([QT, P], F16, tag="o")
                nc.vector.tensor_scalar_mul(o_sb[:, 0:NBLK], z0, rl)
                nc.sync.dma_start(
                    out=out[i * QT:(i + 1) * QT, 0:NBLK], in_=o_sb[:, 0:NBLK])
                nc.vector.tensor_scalar_mul(o_sb[:, NBLK:P], z1, rl)
                nc.scalar.dma_start(
                    out=out[i * QT:(i + 1) * QT, NBLK:P], in_=o_sb[:, NBLK:P])

            # 3-stage pipeline: U(i) then scores(i+1) then uT/Z(i), so both
            # the exp latency of row i+1 and the u-copy latency of row i
            # hide behind PE work. The last two rows' scores (ext 4 and 2)
            # are too short to hide their own softmax, so they are issued
            # two ahead, during row 5's U/Z.
            sc = {}
            sc[0] = emit_scores(0)
            for idx in range(NQT):
                u_sb = emit_u(idx, sc[idx][0])
                if idx + 1 <= 5:
                    sc[idx + 1] = emit_scores(idx + 1)
                elif idx + 1 == 6:
                    sc[6] = emit_scores(6)
                    sc[7] = emit_scores(7)
                emit_z(idx, u_sb, sc[idx][1])

    nc.compile()
    return nc


def _tiles_for_core(c):
    """Global 128-row query-tile indices, in program order i=0..7."""
    return [(15 - 2 * i) if c < 4 else (14 - 2 * i) for i in range(NQT)]


def _host_prep(inputs, Wq, Wk, Wv):
    x = np.asarray(inputs, dtype=np.float32)
    Wqf = np.asarray(Wq, dtype=np.float32)
    Wkf = np.asarray(Wk, dtype=np.float32)
    # scores = x (Wq^T Wk) x^T; device stationary wants the transpose
    ATm = np.ascontiguousarray((Wkf.T @ Wqf).astype(np.float16))
    WvT = np.ascontiguousarray(
        np.asarray(Wv, dtype=np.float32).T.astype(np.float16))

    qi = np.arange(QT)[:, None]
    ki = np.arange(128)[None, :]
    tri = np.where(qi >= ki, 0.0, NEG).astype(np.float32)
    mask_hi = np.concatenate([np.zeros((QT, 128), np.float32), tri], axis=1)
    mask_lo = np.concatenate(
        [tri, np.full((QT, 128), NEG, np.float32)], axis=1)

    in_maps = []
    xT_cache = {}
    for c in range(N_CORES):
        b = c % 4
        if b not in xT_cache:
            xT_cache[b] = np.ascontiguousarray(x[b].T.astype(np.float16))
        xTb = xT_cache[b]
        cols = np.concatenate(
            [xTb[:, t * QT:(t + 1) * QT] for t in _tiles_for_core(c)], axis=1)
        in_maps.append({
            "xT": xTb,
            "xn": np.ascontiguousarray(x[b].astype(np.float16)),
            "xqcols": np.ascontiguousarray(cols),
            "AT": ATm,
            "WvT": WvT,
            "mask": mask_hi if c < 4 else mask_lo,
            "ident": np.eye(128, dtype=np.float16),
        })
    return in_maps


def _host_gather(results):
    Z = np.empty((BATCH, SEQ, P), dtype=np.float32)
    for c in range(N_CORES):
        b = c % 4
        o = results[c]["out"]
        for i, t in enumerate(_tiles_for_core(c)):
            Z[b, t * QT:(t + 1) * QT, :] = o[i * QT:(i + 1) * QT, :]
    return Z


_NC_CACHE = None


def kernel(inputs, Wq, Wk, Wv):
    global _NC_CACHE
    if _NC_CACHE is None:
        _NC_CACHE = build_program()
    in_maps = _host_prep(inputs, Wq, Wk, Wv)
    # The first execution after a fresh compile occasionally hits a
    # transient NRT_EXEC_UNIT_UNRECOVERABLE; a retry reliably succeeds.
    last_err = None
    Z = None
    for _ in range(3):
        try:
            res = run_bass_kernel_spmd(
                _NC_CACHE, in_maps, list(range(N_CORES)))
            Z = _host_gather(res.results)
            if np.isfinite(Z).all():
                return Z
        except Exception as e:  # noqa: BLE001
            last_err = e
    if Z is not None:
        return Z
    raise last_err



# revision 30
# speedup vs baseline: 1.2290x; 1.0174x over previous
"""Causal self-attention (single head) on 8 Trainium2 NeuronCores.

Sharding: 8 cores = 4 batches x 2 query-tile parity sets. Core c handles
batch (c % 4). Cores 0-3 take query tiles t in {15,13,...,1} (128 rows
each), cores 4-7 take t in {14,12,...,0}. Attention iteration i=0..7 uses
a fixed causal extent E(i) = 16-2i k-tiles, so a single SPMD program
serves all cores; even-parity cores waste one fully-masked k-tile per
iteration.

Host passes x.T (plus the core's own query columns pre-gathered) and W.T
per core so the device never transposes inputs; operands are fp16 with
f32 PSUM accumulation. Softmax skips max-subtraction (scores/32 stay in a
safe exp range) and gets row sums free via the activation accum_out. All
operands stay SBUF-resident.

Schedule: the G = A x^T critical DMA (at + xk) is split across both DMA
queues in big deadline-ordered chunks; everything else queues behind.
Attention runs largest-extent first through a 3-stage software pipeline
(U(i) -> scores(i+1) -> uT/Z(i)) with PE transposes issued two tiles
ahead of their consumers, so softmax and copy latencies hide behind PE
work and the kernel tail ends on the smallest row. Output is stored as
fp16 (halves store DMA); the host gather upcasts to f32.
"""

import sys

for _p in ("/opt/trn_rl_repo", "/root/.axon_site/_ro/trn_rl_repo"):
    if _p not in sys.path:
        sys.path.append(_p)

import numpy as np

import concourse.bass as bass  # noqa: F401
import concourse.mybir as mybir
import concourse.tile as tile
from concourse import bacc
from concourse.bass_utils import run_bass_kernel_spmd

F32 = mybir.dt.float32
F16 = mybir.dt.float16

BATCH, SEQ, D, P = 4, 2048, 1024, 1024
N_CORES = 8
QT = 128          # query tile rows
KTL = 128         # key tile
NBLK = 512        # matmul moving free dim
ND = D // 128     # 8 d-tiles
NP = P // 128     # 8 p-tiles
NKT = SEQ // KTL  # 16 k-tiles
NQT = 8           # q-tiles per core
SCALE = 1.0 / float(np.sqrt(P))
NEG = -1e9


def _extent(i):
    return 16 - 2 * i


def _chunks(width):
    out = []
    w = width
    while w >= NBLK:
        out.append(NBLK)
        w -= NBLK
    if w:
        assert w == 256, w
        out.append(256)
    return out


def build_program():
    nc = bacc.Bacc("TRN2", target_bir_lowering=False)

    xT = nc.dram_tensor("xT", [D, SEQ], F16, kind="ExternalInput")
    xn = nc.dram_tensor("xn", [SEQ, D], F16, kind="ExternalInput")
    xq_in = nc.dram_tensor("xqcols", [D, NQT * QT], F16, kind="ExternalInput")
    AT = nc.dram_tensor("AT", [D, D], F16, kind="ExternalInput")
    WvT = nc.dram_tensor("WvT", [D, P], F16, kind="ExternalInput")
    mask = nc.dram_tensor("mask", [QT, 256], F32, kind="ExternalInput")
    ident_in = nc.dram_tensor("ident", [128, 128], F16, kind="ExternalInput")
    out = nc.dram_tensor("out", [NQT * QT, P], F16, kind="ExternalOutput")

    # [128, dt, cols] views (partition dim first); full-row reads keep the
    # DMA's contiguous runs at row length (2-4KB), not a sliced 1KB.
    xT_r = xT.rearrange("(dt dp) s -> dp dt s", dp=128)
    xn_r = xn.rearrange("(kt kp) d -> kp kt d", kp=128)
    xq_r = xq_in.rearrange("(dt dp) q -> dp dt q", dp=128)
    at_r = AT.rearrange("(dt dp) d -> dp dt d", dp=128)
    wv_r = WvT.rearrange("(dt dp) p -> dp dt p", dp=128)

    with tile.TileContext(nc) as tc:
        with (
            tc.tile_pool(name="resident", bufs=1) as resident,
            tc.tile_pool(name="wrow", bufs=3) as wrow,
            tc.tile_pool(name="small", bufs=6) as small,
            tc.tile_pool(name="outp", bufs=2) as outp,
            tc.tile_pool(name="p0psum", bufs=3, space="PSUM") as p0psum,
            tc.tile_pool(name="zpsum", bufs=3, space="PSUM") as zpsum,
            tc.tile_pool(name="tpsum", bufs=2, space="PSUM") as tpsum,
        ):
            kt_sb = resident.tile([128, NP, SEQ], F16)    # G = A x^T [d, k]
            xn_all = resident.tile([128, NKT, D], F16)    # x natural [k, d]
            xq_all = resident.tile([128, ND, NQT * QT], F16)  # x.T q-cols
            xk_all = resident.tile([128, ND, SEQ], F16)   # x.T resident
            at_sb = resident.tile([128, ND, D], F16)      # A^T = Wk^T Wq
            wv_sb = resident.tile([128, ND, P], F16)
            mask_sb = resident.tile([QT, 256], F32)
            ident = resident.tile([128, 128], F16)
            cbias = resident.tile([QT, 1], F32)
            nc.vector.memset(cbias, -4.0)

            # startup loads. The PE queue executes G first, and G's kb-th
            # block needs at (2MB) + xk kb chunk (1MB). DMA issue costs
            # ~0.6us engine time each with ~4 in flight per queue, so use
            # FEW BIG transfers: both queues carry the G-critical path
            # (sync d0-3, scalar d4-7) in deadline order; xq/mask/xn/wv
            # are needed only when attention starts (~G end) and queue
            # strictly behind.
            nc.sync.dma_start(out=at_sb[:, 0:2, :], in_=at_r[:, 0:2, :])
            nc.scalar.dma_start(out=at_sb[:, 4:6, :], in_=at_r[:, 4:6, :])
            nc.sync.dma_start(out=at_sb[:, 2:4, :], in_=at_r[:, 2:4, :])
            nc.scalar.dma_start(out=at_sb[:, 6:ND, :], in_=at_r[:, 6:ND, :])
            for kb in range(SEQ // NBLK):
                s = slice(kb * NBLK, (kb + 1) * NBLK)
                nc.sync.dma_start(out=xk_all[:, 0:4, s], in_=xT_r[:, 0:4, s])
                nc.scalar.dma_start(
                    out=xk_all[:, 4:ND, s], in_=xT_r[:, 4:ND, s])
            nc.scalar.dma_start(out=mask_sb, in_=mask[:, :])
            nc.scalar.dma_start(out=ident, in_=ident_in[:, :])
            nc.scalar.dma_start(out=xq_all, in_=xq_r)
            nc.sync.dma_start(
                out=xn_all[:, 0:NKT // 2, :], in_=xn_r[:, 0:NKT // 2, :])
            nc.sync.dma_start(
                out=xn_all[:, NKT // 2:NKT, :], in_=xn_r[:, NKT // 2:NKT, :])
            nc.scalar.dma_start(out=wv_sb, in_=wv_r)

            # --- G = A x^T and V production ---
            for kb in range(SEQ // NBLK):
                for pt in range(NP):
                    ps = p0psum.tile([128, NBLK], F32, tag="p0")
                    for d in range(ND):
                        nc.tensor.matmul(
                            ps,
                            at_sb[:, d, pt * 128:(pt + 1) * 128],
                            xk_all[:, d, kb * NBLK:(kb + 1) * NBLK],
                            start=(d == 0),
                            stop=(d == ND - 1),
                        )
                    dst = kt_sb[:, pt, kb * NBLK:(kb + 1) * NBLK]
                    if pt % 2 == 0:
                        nc.scalar.copy(dst, ps)
                    else:
                        nc.vector.tensor_copy(dst, ps)

            # --- attention, largest extent first, software-pipelined:
            # scores(i+1) is issued on the PE queue before U/Z(i), so the
            # softmax (vector mask + scalar exp) of each row hides behind
            # PE work instead of bubbling, and the kernel tail ends on the
            # smallest row (ext=2). ---
            def emit_scores(i):
                ext = _extent(i)
                width = ext * KTL
                chunks = _chunks(width)
                s_ps = []
                off = 0
                for cw in chunks:
                    ps_full = p0psum.tile([QT, NBLK], F32, tag="p0")
                    ps = ps_full[:, :cw]
                    for pt in range(NP):
                        nc.tensor.matmul(
                            ps,
                            xq_all[:, pt, i * QT:(i + 1) * QT],
                            kt_sb[:, pt, off:off + cw],
                            start=(pt == 0),
                            stop=(pt == NP - 1),
                        )
                    s_ps.append((ps, off, cw))
                    off += cw

                # additive causal mask on the last 256 columns of the row
                last_ps, _, last_w = s_ps[-1]
                nc.vector.tensor_add(
                    last_ps[:, last_w - 256:last_w],
                    last_ps[:, last_w - 256:last_w],
                    mask_sb,
                )

                # exp((s + m) * scale) -> fp16 weights row; row sums free
                w_sb = wrow.tile([QT, width], F16, tag="w")
                lparts = small.tile([QT, len(chunks)], F32, tag="lp")
                for ci, (ps, off_c, cw) in enumerate(s_ps):
                    nc.scalar.activation(
                        w_sb[:, off_c:off_c + cw],
                        ps,
                        mybir.ActivationFunctionType.Exp,
                        scale=SCALE,
                        bias=cbias,
                        accum_out=lparts[:, ci:ci + 1],
                    )

                lsum = small.tile([QT, 1], F32, tag="ls")
                nc.vector.reduce_sum(lsum, lparts, axis=mybir.AxisListType.X)
                rl = small.tile([QT, 1], F32, tag="rl")
                nc.vector.reciprocal(rl, lsum)
                return w_sb, rl

            def emit_u(i, w_sb):
                ext = _extent(i)
                # U = W x  (transpose each weight block on PE, two k-tiles
                # ahead of the consuming matmuls)
                u0 = zpsum.tile([QT, NBLK], F32, tag="z")
                u1 = zpsum.tile([QT, NBLK], F32, tag="z")
                wTs = {}

                def wtp_one(kt):
                    tp = tpsum.tile([128, 128], F16, tag="tp")
                    nc.tensor.transpose(
                        tp, w_sb[:, kt * 128:(kt + 1) * 128], ident)
                    wT = small.tile([128, 128], F16, tag="wT")
                    nc.vector.tensor_copy(wT, tp)
                    wTs[kt] = wT

                wtp_one(0)
                if ext > 1:
                    wtp_one(1)
                for kt in range(ext):
                    if kt + 2 < ext:
                        wtp_one(kt + 2)
                    nc.tensor.matmul(
                        u0, wTs[kt], xn_all[:, kt, 0:NBLK],
                        start=(kt == 0), stop=(kt == ext - 1),
                    )
                    nc.tensor.matmul(
                        u1, wTs[kt], xn_all[:, kt, NBLK:D],
                        start=(kt == 0), stop=(kt == ext - 1),
                    )
                u_sb = wrow.tile([QT, D], F16, tag="u")
                nc.scalar.copy(u_sb[:, 0:NBLK], u0)
                nc.vector.tensor_copy(u_sb[:, NBLK:D], u1)
                return u_sb

            def emit_z(i, u_sb, rl):
                # Z = U Wv^T  (U transposed per d-tile on PE, two tiles
                # ahead of the Z accumulation)
                uT_sb = small.tile([128, ND, 128], F16, tag="uT")

                def tp_one(dt):
                    tpu = tpsum.tile([128, 128], F16, tag="tp")
                    nc.tensor.transpose(
                        tpu, u_sb[:, dt * 128:(dt + 1) * 128], ident)
                    nc.vector.tensor_copy(uT_sb[:, dt, :], tpu)

                tp_one(0)
                tp_one(1)
                z0 = zpsum.tile([QT, NBLK], F32, tag="z")
                z1 = zpsum.tile([QT, NBLK], F32, tag="z")
                for dt in range(ND):
                    if dt + 2 < ND:
                        tp_one(dt + 2)
                    nc.tensor.matmul(
                        z0, uT_sb[:, dt, :], wv_sb[:, dt, 0:NBLK],
                        start=(dt == 0), stop=(dt == ND - 1),
                    )
                    nc.tensor.matmul(
                        z1, uT_sb[:, dt, :], wv_sb[:, dt, NBLK:P],
                        start=(dt == 0), stop=(dt == ND - 1),
                    )

                o_sb = outp.tile([QT, P], F16, tag="o")
                nc.vector.tensor_scalar_mul(o_sb[:, 0:NBLK], z0, rl)
                nc.sync.dma_start(
                    out=out[i * QT:(i + 1) * QT, 0:NBLK], in_=o_sb[:, 0:NBLK])
                nc.vector.tensor_scalar_mul(o_sb[:, NBLK:P], z1, rl)
                nc.scalar.dma_start(
                    out=out[i * QT:(i + 1) * QT, NBLK:P], in_=o_sb[:, NBLK:P])

            # 3-stage pipeline: U(i) then scores(i+1) then uT/Z(i), so both
            # the exp latency of row i+1 and the u-copy latency of row i
            # hide behind PE work. The last two rows' scores (ext 4 and 2)
            # are too short to hide their own softmax, so they are issued
            # two ahead, during row 5's U/Z.
            sc = {}
            sc[0] = emit_scores(0)
            for idx in range(NQT):
                u_sb = emit_u(idx, sc[idx][0])
                if idx + 1 <= 5:
                    sc[idx + 1] = emit_scores(idx + 1)
                elif idx + 1 == 6:
                    sc[6] = emit_scores(6)
                    sc[7] = emit_scores(7)
                emit_z(idx, u_sb, sc[idx][1])

    nc.compile()
    return nc


def _tiles_for_core(c):
    """Global 128-row query-tile indices, in program order i=0..7."""
    return [(15 - 2 * i) if c < 4 else (14 - 2 * i) for i in range(NQT)]


def _host_prep(inputs, Wq, Wk, Wv):
    x = np.asarray(inputs, dtype=np.float32)
    Wqf = np.asarray(Wq, dtype=np.float32)
    Wkf = np.asarray(Wk, dtype=np.float32)
    # scores = x (Wq^T Wk) x^T; device stationary wants the transpose
    ATm = np.ascontiguousarray((Wkf.T @ Wqf).astype(np.float16))
    WvT = np.ascontiguousarray(
        np.asarray(Wv, dtype=np.float32).T.astype(np.float16))

    qi = np.arange(QT)[:, None]
    ki = np.arange(128)[None, :]
    tri = np.where(qi >= ki, 0.0, NEG).astype(np.float32)
    mask_hi = np.concatenate([np.zeros((QT, 128), np.float32), tri], axis=1)
    mask_lo = np.concatenate(
        [tri, np.full((QT, 128), NEG, np.float32)], axis=1)

    in_maps = []
    xT_cache = {}
    for c in range(N_CORES):
        b = c % 4
        if b not in xT_cache:
            xT_cache[b] = np.ascontiguousarray(x[b].T.astype(np.float16))
        xTb = xT_cache[b]
        cols = np.concatenate(
            [xTb[:, t * QT:(t + 1) * QT] for t in _tiles_for_core(c)], axis=1)
        in_maps.append({
            "xT": xTb,
            "xn": np.ascontiguousarray(x[b].astype(np.float16)),
            "xqcols": np.ascontiguousarray(cols),
            "AT": ATm,
            "WvT": WvT,
            "mask": mask_hi if c < 4 else mask_lo,
            "ident": np.eye(128, dtype=np.float16),
        })
    return in_maps


def _host_gather(results):
    Z = np.empty((BATCH, SEQ, P), dtype=np.float32)
    for c in range(N_CORES):
        b = c % 4
        o = results[c]["out"]
        for i, t in enumerate(_tiles_for_core(c)):
            Z[b, t * QT:(t + 1) * QT, :] = o[i * QT:(i + 1) * QT, :]
    return Z


_NC_CACHE = None


def kernel(inputs, Wq, Wk, Wv):
    global _NC_CACHE
    if _NC_CACHE is None:
        _NC_CACHE = build_program()
    in_maps = _host_prep(inputs, Wq, Wk, Wv)
    # The first execution after a fresh compile occasionally hits a
    # transient NRT_EXEC_UNIT_UNRECOVERABLE; a retry reliably succeeds.
    last_err = None
    Z = None
    for _ in range(3):
        try:
            res = run_bass_kernel_spmd(
                _NC_CACHE, in_maps, list(range(N_CORES)))
            Z = _host_gather(res.results)
            if np.isfinite(Z).all():
                return Z
        except Exception as e:  # noqa: BLE001
            last_err = e
    if Z is not None:
        return Z
    raise last_err



# revision 31
# speedup vs baseline: 1.2446x; 1.0127x over previous
"""Causal self-attention (single head) on 8 Trainium2 NeuronCores.

Sharding: 8 cores = 4 batches x 2 query-tile parity sets. Core c handles
batch (c % 4). Cores 0-3 take query tiles t in {15,13,...,1} (128 rows
each), cores 4-7 take t in {14,12,...,0}. Attention iteration i=0..7 uses
a fixed causal extent E(i) = 16-2i k-tiles, so a single SPMD program
serves all cores; even-parity cores waste one fully-masked k-tile per
iteration.

Host passes x.T (plus the core's own query columns pre-gathered) and W.T
per core so the device never transposes inputs; operands are fp16 with
f32 PSUM accumulation. Softmax skips max-subtraction (scores/32 stay in a
safe exp range) and gets row sums free via the activation accum_out. All
operands stay SBUF-resident.

Schedule: the G = A x^T critical DMA (at + xk) is split across both DMA
queues in big deadline-ordered chunks; everything else queues behind.
Attention runs largest-extent first through a 3-stage software pipeline
(U(i) -> scores(i+1) -> uT/Z(i)) with PE transposes issued two tiles
ahead of their consumers, so softmax and copy latencies hide behind PE
work and the kernel tail ends on the smallest row. Output is stored as
fp16 (halves store DMA); the host gather upcasts to f32.
"""

import sys

for _p in ("/opt/trn_rl_repo", "/root/.axon_site/_ro/trn_rl_repo"):
    if _p not in sys.path:
        sys.path.append(_p)

import numpy as np

import concourse.bass as bass  # noqa: F401
import concourse.mybir as mybir
import concourse.tile as tile
from concourse import bacc
from concourse.bass_utils import run_bass_kernel_spmd

F32 = mybir.dt.float32
F16 = mybir.dt.float16

BATCH, SEQ, D, P = 4, 2048, 1024, 1024
N_CORES = 8
QT = 128          # query tile rows
KTL = 128         # key tile
NBLK = 512        # matmul moving free dim
ND = D // 128     # 8 d-tiles
NP = P // 128     # 8 p-tiles
NKT = SEQ // KTL  # 16 k-tiles
NQT = 8           # q-tiles per core
SCALE = 1.0 / float(np.sqrt(P))
NEG = -1e9


def _extent(i):
    return 16 - 2 * i


def _chunks(width):
    out = []
    w = width
    while w >= NBLK:
        out.append(NBLK)
        w -= NBLK
    if w:
        assert w == 256, w
        out.append(256)
    return out


def build_program():
    nc = bacc.Bacc("TRN2", target_bir_lowering=False)

    xT = nc.dram_tensor("xT", [D, SEQ], F16, kind="ExternalInput")
    xn = nc.dram_tensor("xn", [SEQ, D], F16, kind="ExternalInput")
    xq_in = nc.dram_tensor("xqcols", [D, NQT * QT], F16, kind="ExternalInput")
    AT = nc.dram_tensor("AT", [D, D], F16, kind="ExternalInput")
    WvT = nc.dram_tensor("WvT", [D, P], F16, kind="ExternalInput")
    mask = nc.dram_tensor("mask", [QT, 256], F32, kind="ExternalInput")
    ident_in = nc.dram_tensor("ident", [128, 128], F16, kind="ExternalInput")
    out = nc.dram_tensor("out", [NQT * QT, P], F16, kind="ExternalOutput")

    # [128, dt, cols] views (partition dim first); full-row reads keep the
    # DMA's contiguous runs at row length (2-4KB), not a sliced 1KB.
    xT_r = xT.rearrange("(dt dp) s -> dp dt s", dp=128)
    xn_r = xn.rearrange("(kt kp) d -> kp kt d", kp=128)
    xq_r = xq_in.rearrange("(dt dp) q -> dp dt q", dp=128)
    at_r = AT.rearrange("(dt dp) d -> dp dt d", dp=128)
    wv_r = WvT.rearrange("(dt dp) p -> dp dt p", dp=128)

    with tile.TileContext(nc) as tc:
        with (
            tc.tile_pool(name="resident", bufs=1) as resident,
            tc.tile_pool(name="wrow", bufs=3) as wrow,
            tc.tile_pool(name="small", bufs=6) as small,
            tc.tile_pool(name="outp", bufs=2) as outp,
            tc.tile_pool(name="p0psum", bufs=3, space="PSUM") as p0psum,
            tc.tile_pool(name="zpsum", bufs=3, space="PSUM") as zpsum,
            tc.tile_pool(name="tpsum", bufs=2, space="PSUM") as tpsum,
        ):
            kt_sb = resident.tile([128, NP, SEQ], F16)    # G = A x^T [d, k]
            xn_all = resident.tile([128, NKT, D], F16)    # x natural [k, d]
            xq_all = resident.tile([128, ND, NQT * QT], F16)  # x.T q-cols
            xk_all = resident.tile([128, ND, SEQ], F16)   # x.T resident
            at_sb = resident.tile([128, ND, D], F16)      # A^T = Wk^T Wq
            wv_sb = resident.tile([128, ND, P], F16)
            mask_sb = resident.tile([QT, 256], F32)
            ident = resident.tile([128, 128], F16)
            cbias = resident.tile([QT, 1], F32)
            nc.vector.memset(cbias, -4.0)

            # PE warmup during the DMA dead zone: the clock ramps over the
            # first ~25us (early matmuls run ~30% slow); burn dummy
            # matmuls on zeroed scratch while the first loads are in
            # flight so real G work starts at full clock. Result is never
            # read.
            warm = resident.tile([128, NBLK], F16)
            nc.vector.memset(warm, 0.0)
            wps = p0psum.tile([128, NBLK], F32, tag="p0")
            NWARM = 24
            for r in range(NWARM):
                nc.tensor.matmul(
                    wps, warm[:, 0:128], warm,
                    start=(r == 0), stop=(r == NWARM - 1))

            # startup loads. The PE queue executes G first, and G's kb-th
            # block needs at (2MB) + xk kb chunk (1MB). DMA issue costs
            # ~0.6us engine time each with ~4 in flight per queue, so use
            # FEW BIG transfers: both queues carry the G-critical path
            # (sync d0-3, scalar d4-7) in deadline order; xq/mask/xn/wv
            # are needed only when attention starts (~G end) and queue
            # strictly behind.
            nc.sync.dma_start(out=at_sb[:, 0:2, :], in_=at_r[:, 0:2, :])
            nc.scalar.dma_start(out=at_sb[:, 4:6, :], in_=at_r[:, 4:6, :])
            nc.sync.dma_start(out=at_sb[:, 2:4, :], in_=at_r[:, 2:4, :])
            nc.scalar.dma_start(out=at_sb[:, 6:ND, :], in_=at_r[:, 6:ND, :])
            for kb in range(SEQ // NBLK):
                s = slice(kb * NBLK, (kb + 1) * NBLK)
                nc.sync.dma_start(out=xk_all[:, 0:4, s], in_=xT_r[:, 0:4, s])
                nc.scalar.dma_start(
                    out=xk_all[:, 4:ND, s], in_=xT_r[:, 4:ND, s])
            nc.scalar.dma_start(out=mask_sb, in_=mask[:, :])
            nc.scalar.dma_start(out=ident, in_=ident_in[:, :])
            nc.scalar.dma_start(out=xq_all, in_=xq_r)
            nc.sync.dma_start(
                out=xn_all[:, 0:NKT // 2, :], in_=xn_r[:, 0:NKT // 2, :])
            nc.sync.dma_start(
                out=xn_all[:, NKT // 2:NKT, :], in_=xn_r[:, NKT // 2:NKT, :])
            nc.scalar.dma_start(out=wv_sb, in_=wv_r)

            # --- G = A x^T and V production ---
            for kb in range(SEQ // NBLK):
                for pt in range(NP):
                    ps = p0psum.tile([128, NBLK], F32, tag="p0")
                    for d in range(ND):
                        nc.tensor.matmul(
                            ps,
                            at_sb[:, d, pt * 128:(pt + 1) * 128],
                            xk_all[:, d, kb * NBLK:(kb + 1) * NBLK],
                            start=(d == 0),
                            stop=(d == ND - 1),
                        )
                    dst = kt_sb[:, pt, kb * NBLK:(kb + 1) * NBLK]
                    if pt % 2 == 0:
                        nc.scalar.copy(dst, ps)
                    else:
                        nc.vector.tensor_copy(dst, ps)

            # --- attention, largest extent first, software-pipelined:
            # scores(i+1) is issued on the PE queue before U/Z(i), so the
            # softmax (vector mask + scalar exp) of each row hides behind
            # PE work instead of bubbling, and the kernel tail ends on the
            # smallest row (ext=2). ---
            def emit_scores(i):
                ext = _extent(i)
                width = ext * KTL
                chunks = _chunks(width)
                s_ps = []
                off = 0
                for cw in chunks:
                    ps_full = p0psum.tile([QT, NBLK], F32, tag="p0")
                    ps = ps_full[:, :cw]
                    for pt in range(NP):
                        nc.tensor.matmul(
                            ps,
                            xq_all[:, pt, i * QT:(i + 1) * QT],
                            kt_sb[:, pt, off:off + cw],
                            start=(pt == 0),
                            stop=(pt == NP - 1),
                        )
                    s_ps.append((ps, off, cw))
                    off += cw

                # additive causal mask on the last 256 columns of the row
                last_ps, _, last_w = s_ps[-1]
                nc.vector.tensor_add(
                    last_ps[:, last_w - 256:last_w],
                    last_ps[:, last_w - 256:last_w],
                    mask_sb,
                )

                # exp((s + m) * scale) -> fp16 weights row; row sums free
                w_sb = wrow.tile([QT, width], F16, tag="w")
                lparts = small.tile([QT, len(chunks)], F32, tag="lp")
                for ci, (ps, off_c, cw) in enumerate(s_ps):
                    nc.scalar.activation(
                        w_sb[:, off_c:off_c + cw],
                        ps,
                        mybir.ActivationFunctionType.Exp,
                        scale=SCALE,
                        bias=cbias,
                        accum_out=lparts[:, ci:ci + 1],
                    )

                lsum = small.tile([QT, 1], F32, tag="ls")
                nc.vector.reduce_sum(lsum, lparts, axis=mybir.AxisListType.X)
                rl = small.tile([QT, 1], F32, tag="rl")
                nc.vector.reciprocal(rl, lsum)
                return w_sb, rl

            def emit_u(i, w_sb):
                ext = _extent(i)
                # U = W x  (transpose each weight block on PE, two k-tiles
                # ahead of the consuming matmuls)
                u0 = zpsum.tile([QT, NBLK], F32, tag="z")
                u1 = zpsum.tile([QT, NBLK], F32, tag="z")
                wTs = {}

                def wtp_one(kt):
                    tp = tpsum.tile([128, 128], F16, tag="tp")
                    nc.tensor.transpose(
                        tp, w_sb[:, kt * 128:(kt + 1) * 128], ident)
                    wT = small.tile([128, 128], F16, tag="wT")
                    nc.vector.tensor_copy(wT, tp)
                    wTs[kt] = wT

                wtp_one(0)
                if ext > 1:
                    wtp_one(1)
                for kt in range(ext):
                    if kt + 2 < ext:
                        wtp_one(kt + 2)
                    nc.tensor.matmul(
                        u0, wTs[kt], xn_all[:, kt, 0:NBLK],
                        start=(kt == 0), stop=(kt == ext - 1),
                    )
                    nc.tensor.matmul(
                        u1, wTs[kt], xn_all[:, kt, NBLK:D],
                        start=(kt == 0), stop=(kt == ext - 1),
                    )
                u_sb = wrow.tile([QT, D], F16, tag="u")
                nc.scalar.copy(u_sb[:, 0:NBLK], u0)
                nc.vector.tensor_copy(u_sb[:, NBLK:D], u1)
                return u_sb

            def emit_z(i, u_sb, rl):
                # Z = U Wv^T  (U transposed per d-tile on PE, two tiles
                # ahead of the Z accumulation)
                uT_sb = small.tile([128, ND, 128], F16, tag="uT")

                def tp_one(dt):
                    tpu = tpsum.tile([128, 128], F16, tag="tp")
                    nc.tensor.transpose(
                        tpu, u_sb[:, dt * 128:(dt + 1) * 128], ident)
                    nc.vector.tensor_copy(uT_sb[:, dt, :], tpu)

                tp_one(0)
                tp_one(1)
                z0 = zpsum.tile([QT, NBLK], F32, tag="z")
                z1 = zpsum.tile([QT, NBLK], F32, tag="z")
                for dt in range(ND):
                    if dt + 2 < ND:
                        tp_one(dt + 2)
                    nc.tensor.matmul(
                        z0, uT_sb[:, dt, :], wv_sb[:, dt, 0:NBLK],
                        start=(dt == 0), stop=(dt == ND - 1),
                    )
                    nc.tensor.matmul(
                        z1, uT_sb[:, dt, :], wv_sb[:, dt, NBLK:P],
                        start=(dt == 0), stop=(dt == ND - 1),
                    )

                o_sb = outp.tile([QT, P], F16, tag="o")
                nc.vector.tensor_scalar_mul(o_sb[:, 0:NBLK], z0, rl)
                nc.sync.dma_start(
                    out=out[i * QT:(i + 1) * QT, 0:NBLK], in_=o_sb[:, 0:NBLK])
                nc.vector.tensor_scalar_mul(o_sb[:, NBLK:P], z1, rl)
                nc.scalar.dma_start(
                    out=out[i * QT:(i + 1) * QT, NBLK:P], in_=o_sb[:, NBLK:P])

            # 3-stage pipeline: U(i) then scores(i+1) then uT/Z(i), so both
            # the exp latency of row i+1 and the u-copy latency of row i
            # hide behind PE work. The last two rows' scores (ext 4 and 2)
            # are too short to hide their own softmax, so they are issued
            # two ahead, during row 5's U/Z.
            sc = {}
            sc[0] = emit_scores(0)
            for idx in range(NQT):
                u_sb = emit_u(idx, sc[idx][0])
                if idx + 1 <= 5:
                    sc[idx + 1] = emit_scores(idx + 1)
                elif idx + 1 == 6:
                    sc[6] = emit_scores(6)
                    sc[7] = emit_scores(7)
                emit_z(idx, u_sb, sc[idx][1])

    nc.compile()
    return nc


def _tiles_for_core(c):
    """Global 128-row query-tile indices, in program order i=0..7."""
    return [(15 - 2 * i) if c < 4 else (14 - 2 * i) for i in range(NQT)]


def _host_prep(inputs, Wq, Wk, Wv):
    x = np.asarray(inputs, dtype=np.float32)
    Wqf = np.asarray(Wq, dtype=np.float32)
    Wkf = np.asarray(Wk, dtype=np.float32)
    # scores = x (Wq^T Wk) x^T; device stationary wants the transpose
    ATm = np.ascontiguousarray((Wkf.T @ Wqf).astype(np.float16))
    WvT = np.ascontiguousarray(
        np.asarray(Wv, dtype=np.float32).T.astype(np.float16))

    qi = np.arange(QT)[:, None]
    ki = np.arange(128)[None, :]
    tri = np.where(qi >= ki, 0.0, NEG).astype(np.float32)
    mask_hi = np.concatenate([np.zeros((QT, 128), np.float32), tri], axis=1)
    mask_lo = np.concatenate(
        [tri, np.full((QT, 128), NEG, np.float32)], axis=1)

    in_maps = []
    xT_cache = {}
    for c in range(N_CORES):
        b = c % 4
        if b not in xT_cache:
            xT_cache[b] = np.ascontiguousarray(x[b].T.astype(np.float16))
        xTb = xT_cache[b]
        cols = np.concatenate(
            [xTb[:, t * QT:(t + 1) * QT] for t in _tiles_for_core(c)], axis=1)
        in_maps.append({
            "xT": xTb,
            "xn": np.ascontiguousarray(x[b].astype(np.float16)),
            "xqcols": np.ascontiguousarray(cols),
            "AT": ATm,
            "WvT": WvT,
            "mask": mask_hi if c < 4 else mask_lo,
            "ident": np.eye(128, dtype=np.float16),
        })
    return in_maps


def _host_gather(results):
    Z = np.empty((BATCH, SEQ, P), dtype=np.float32)
    for c in range(N_CORES):
        b = c % 4
        o = results[c]["out"]
        for i, t in enumerate(_tiles_for_core(c)):
            Z[b, t * QT:(t + 1) * QT, :] = o[i * QT:(i + 1) * QT, :]
    return Z


_NC_CACHE = None


def kernel(inputs, Wq, Wk, Wv):
    global _NC_CACHE
    if _NC_CACHE is None:
        _NC_CACHE = build_program()
    in_maps = _host_prep(inputs, Wq, Wk, Wv)
    # The first execution after a fresh compile occasionally hits a
    # transient NRT_EXEC_UNIT_UNRECOVERABLE; a retry reliably succeeds.
    last_err = None
    Z = None
    for _ in range(3):
        try:
            res = run_bass_kernel_spmd(
                _NC_CACHE, in_maps, list(range(N_CORES)))
            Z = _host_gather(res.results)
            if np.isfinite(Z).all():
                return Z
        except Exception as e:  # noqa: BLE001
            last_err = e
    if Z is not None:
        return Z
    raise last_err

